# revision 1
# baseline (speedup 1.0000x reference)
"""Multi-head attention (RoPE, non-causal) on 8 Trainium2 NeuronCores.

Problem: x[4,2048,2048] fp32; wq/wk/wv/wo [2048,2048]; biases [2048].
  q,k,v = x@w.T+b per 16 heads of dim 128; rope(q,k); softmax(q k^T/sqrt(128));
  out = (attn@v)@wo.T + bo.

Sharding: core c = 2*b + g -> batch b, head-group g (8 heads each).
Each core computes a partial output (its 8 heads) for its batch over the full
sequence; the host sums the pair partials (the wo contraction splits cleanly
over head groups) and adds bo_eff = bo + wo@bv (the V-bias folds out exactly
because softmax rows sum to 1).

Device program (SPMD, one NEFF, bf16 matmul operands / fp32 accumulation):
  V phase: V for all 8 local heads in natural [t, dh] layout
      (xT-chunk stationary, wv moving), spilled to DRAM as bf16.
  Fused per-head loop: Q^T/K^T projection (w-chunk stationary, xT moving)
      -> DVE bias+scale -> RoPE (DMA rotate-half + DVE mul/mul/add, bf16 out,
      stays in SBUF) -> flash-style attention over t-chunks in the transposed
      scores orientation [t, s]: scores matmul -> ACT exp (bf16) -> bf16 DVE
      partial sums for the softmax denominator + ctx^T accumulation in PSUM.
      Denominator finished with a ones-vector matmul (cross-partition sum),
      broadcast via a DRAM-bounce stride-0 DMA, applied with DVE.
      No max-subtraction: |scores| <= ~15 so exp is fp32-safe.
  P3: out[s, :] = sum_c ctxT_c^T @ woT_c over the core's 8 head-chunks.
"""

import sys

if "/opt/trn_rl_repo" not in sys.path:
    sys.path.insert(0, "/opt/trn_rl_repo")

import ml_dtypes
import numpy as np

import concourse.bass as bass
import concourse.tile as tile
from concourse import bacc, mybir
from concourse.bass_utils import run_bass_kernel_spmd

F32 = mybir.dt.float32
BF16 = mybir.dt.bfloat16
NPBF = ml_dtypes.bfloat16

B, S, D = 4, 2048, 2048
H = 16
DH = 128
HL = 8  # heads per core
KO = D // 128  # 16 k-chunks
TB = S // 128  # 16 t-chunks
ROPE_THETA = 10000.0
QSCALE = 1.0 / np.sqrt(DH)

_NC_CACHE = {}


def build_nc():
    nc = bacc.Bacc()

    xt_d = nc.declare_dram_parameter("xt", [KO, 128, S], BF16, isOutput=False)
    wq_d = nc.declare_dram_parameter("wq", [HL, KO, 128, 128], BF16, isOutput=False)
    wk_d = nc.declare_dram_parameter("wk", [HL, KO, 128, 128], BF16, isOutput=False)
    wv_d = nc.declare_dram_parameter("wv", [KO, 128, HL * DH], BF16, isOutput=False)
    wo_d = nc.declare_dram_parameter("wo", [HL, 128, D], BF16, isOutput=False)
    cos_d = nc.declare_dram_parameter("cosT", [128, S], F32, isOutput=False)
    sin_d = nc.declare_dram_parameter("sinS", [128, S], F32, isOutput=False)
    bq_d = nc.declare_dram_parameter("bq", [128, HL], F32, isOutput=False)
    bk_d = nc.declare_dram_parameter("bk", [128, HL], F32, isOutput=False)
    out_d = nc.declare_dram_parameter("out", [S, D], F32, isOutput=True)

    v_d = nc.dram_tensor("v_spill", [TB, 128, HL * DH], BF16)
    ct_d = nc.dram_tensor("ct_spill", [HL, 128, S], BF16)
    den_d = nc.dram_tensor("den_bounce", [HL, 1, S], F32)

    with tile.TileContext(nc) as tc:
        with (
            tc.tile_pool(name="xt_pool", bufs=1) as xt_pool,
            tc.tile_pool(name="const_pool", bufs=1) as const_pool,
            tc.tile_pool(name="psum_main", bufs=1, space="PSUM") as psum_main,
        ):
            xt_sb = xt_pool.tile([128, KO, S], BF16)
            nc.sync.dma_start(out=xt_sb[:], in_=xt_d[:].rearrange("k p s -> p k s"))

            cos_sb = const_pool.tile([128, S], F32)
            sin_sb = const_pool.tile([128, S], F32)
            bq_sb = const_pool.tile([128, HL], F32)
            bk_sb = const_pool.tile([128, HL], F32)
            ones_sb = const_pool.tile([128, 1], BF16)
            nc.sync.dma_start(out=cos_sb[:], in_=cos_d[:])
            nc.sync.dma_start(out=sin_sb[:], in_=sin_d[:])
            nc.sync.dma_start(out=bq_sb[:], in_=bq_d[:])
            nc.sync.dma_start(out=bk_sb[:], in_=bk_d[:])
            nc.vector.memset(ones_sb[:], 1.0)

            # ---------------- V phase: natural [t, dh] layout, all heads ----
            with (
                tc.tile_pool(name="wv_pool", bufs=1) as wv_pool,
                tc.tile_pool(name="vout_pool", bufs=3) as vout_pool,
            ):
                wv_sb = wv_pool.tile([128, KO, HL * DH], BF16)
                nc.sync.dma_start(
                    out=wv_sb[:], in_=wv_d[:].rearrange("k p m -> p k m")
                )
                for tb in range(TB):
                    for nf in range(2):
                        vps = psum_main.tile([128, 512], F32, tag="qk", bufs=2)
                        for ko in range(KO):
                            nc.tensor.matmul(
                                vps[:],
                                xt_sb[:, ko, tb * 128 : (tb + 1) * 128],
                                wv_sb[:, ko, nf * 512 : (nf + 1) * 512],
                                start=(ko == 0),
                                stop=(ko == KO - 1),
                            )
                        vsb = vout_pool.tile([128, 512], BF16)
                        nc.vector.tensor_copy(out=vsb[:], in_=vps[:])
                        nc.sync.dma_start(
                            out=v_d[tb, :, nf * 512 : (nf + 1) * 512], in_=vsb[:]
                        )

            # -------- fused per-head: QK projection + rope + attention ------
            with (
                tc.tile_pool(name="w_pool", bufs=2) as w_pool,
                tc.tile_pool(name="qs_pool", bufs=2) as qs_pool,
                tc.tile_pool(name="rot_pool", bufs=2) as rot_pool,
                tc.tile_pool(name="qb_pool", bufs=2) as qb_pool,
                tc.tile_pool(name="v2_pool", bufs=2) as v2_pool,
                tc.tile_pool(name="et_pool", bufs=4) as et_pool,
                tc.tile_pool(name="den_pool", bufs=2) as den_pool,
                tc.tile_pool(name="norm_pool", bufs=2) as norm_pool,
            ):
                for h in range(HL):
                    # --- Q^T and K^T for head h (rope'd, bf16, in SBUF)
                    qkb = []
                    for w_d, b_sb, scale in (
                        (wq_d, bq_sb, QSCALE),
                        (wk_d, bk_sb, 1.0),
                    ):
                        w_sb = w_pool.tile([128, KO, 128], BF16)
                        nc.sync.dma_start(
                            out=w_sb[:], in_=w_d[h].rearrange("k p m -> p k m")
                        )
                        qs = qs_pool.tile([128, S], F32)
                        for sh in range(2):  # s in two 1024 halves
                            qps = psum_main.tile([128, 1024], F32, tag="qk", bufs=2)
                            for ko in range(KO):
                                for n in range(2):
                                    nc.tensor.matmul(
                                        qps[:, n * 512 : (n + 1) * 512],
                                        w_sb[:, ko, :],
                                        xt_sb[
                                            :,
                                            ko,
                                            sh * 1024
                                            + n * 512 : sh * 1024
                                            + (n + 1) * 512,
                                        ],
                                        start=(ko == 0),
                                        stop=(ko == KO - 1),
                                    )
                            # qs = psum*scale + bias (per-partition)
                            nc.vector.tensor_scalar(
                                out=qs[:, sh * 1024 : (sh + 1) * 1024],
                                in0=qps[:],
                                scalar1=scale,
                                scalar2=b_sb[:, h : h + 1],
                                op0=mybir.AluOpType.mult,
                                op1=mybir.AluOpType.add,
                            )
                        # rope: q' = q*cos + rot(q)*sinS (sinS sign-folded)
                        rot = rot_pool.tile([128, S], F32)
                        nc.sync.dma_start(out=rot[0:64, :], in_=qs[64:128, :])
                        nc.sync.dma_start(out=rot[64:128, :], in_=qs[0:64, :])
                        nc.vector.tensor_mul(out=qs[:], in0=qs[:], in1=cos_sb[:])
                        nc.vector.tensor_mul(out=rot[:], in0=rot[:], in1=sin_sb[:])
                        qb = qb_pool.tile([128, S], BF16)
                        nc.vector.tensor_add(out=qb[:], in0=qs[:], in1=rot[:])
                        qkb.append(qb)
                    qt_sb, kt_sb = qkb

                    v_sb = v2_pool.tile([128, TB, DH], BF16)
                    nc.sync.dma_start(
                        out=v_sb[:],
                        in_=v_d[:, :, h * DH : (h + 1) * DH].rearrange(
                            "t p m -> p t m"
                        ),
                    )

                    # --- attention over t-chunks (scoresT orientation)
                    ctx_ps = psum_main.tile([128, S], F32, tag="ctx", bufs=1)
                    pden = den_pool.tile([128, S], BF16, tag="pden", bufs=2)
                    for tb in range(TB):
                        et = et_pool.tile([128, S], BF16)
                        for sh in range(2):
                            sc = psum_main.tile([128, 1024], F32, tag="qk", bufs=2)
                            for n in range(2):
                                nc.tensor.matmul(
                                    sc[:, n * 512 : (n + 1) * 512],
                                    kt_sb[:, tb * 128 : (tb + 1) * 128],
                                    qt_sb[
                                        :,
                                        sh * 1024 + n * 512 : sh * 1024 + (n + 1) * 512,
                                    ],
                                    start=True,
                                    stop=True,
                                )
                            nc.scalar.activation(
                                out=et[:, sh * 1024 : (sh + 1) * 1024],
                                in_=sc[:],
                                func=mybir.ActivationFunctionType.Exp,
                            )
                        if tb == 0:
                            nc.vector.tensor_copy(out=pden[:], in_=et[:])
                        else:
                            nc.vector.tensor_add(out=pden[:], in0=pden[:], in1=et[:])
                        for n in range(4):
                            nc.tensor.matmul(
                                ctx_ps[:, n * 512 : (n + 1) * 512],
                                v_sb[:, tb, :],
                                et[:, n * 512 : (n + 1) * 512],
                                start=(tb == 0),
                                stop=(tb == TB - 1),
                            )

                    # --- denominator: cross-partition sum via ones-matmul
                    den_sb = den_pool.tile([1, S], F32, tag="den", bufs=1)
                    for n in range(4):
                        dps = psum_main.tile([1, 512], F32, tag="qk", bufs=2)
                        nc.tensor.matmul(
                            dps[:],
                            ones_sb[:],
                            pden[:, n * 512 : (n + 1) * 512],
                            start=True,
                            stop=True,
                        )
                        nc.scalar.copy(
                            out=den_sb[:, n * 512 : (n + 1) * 512], in_=dps[:]
                        )
                    # fast PSUM release: copy unnormalized ctx^T to SBUF
                    cu = norm_pool.tile([128, S], F32, tag="cu", bufs=1)
                    nc.vector.tensor_copy(out=cu[:], in_=ctx_ps[:])
                    # broadcast den across partitions via DRAM bounce
                    nc.sync.dma_start(out=den_d[h], in_=den_sb[:])
                    bc = norm_pool.tile([128, S], F32, tag="bc")
                    den_ap = den_d[h]
                    bcast_src = bass.AP(
                        tensor=den_ap.tensor,
                        offset=den_ap.offset,
                        ap=[[0, 128]] + list(den_ap.ap[1:]),
                    )
                    nc.sync.dma_start(out=bc[:], in_=bcast_src)
                    nc.vector.reciprocal(out=bc[:], in_=bc[:])
                    ct_sb = norm_pool.tile([128, S], BF16, tag="ct")
                    nc.vector.tensor_mul(out=ct_sb[:], in0=cu[:], in1=bc[:])
                    nc.sync.dma_start(out=ct_d[h], in_=ct_sb[:])

        # ---------------- P3: output projection (partial) ----------------
        with (
            tc.tile_pool(name="wo_pool", bufs=1) as wo_pool,
            tc.tile_pool(name="ct_pool", bufs=2) as ct_pool,
            tc.tile_pool(name="out_pool", bufs=2) as out_pool,
            tc.tile_pool(name="psum_p3", bufs=8, space="PSUM") as psum_p3,
        ):
            wo_sb = wo_pool.tile([128, HL, D], BF16)
            nc.sync.dma_start(out=wo_sb[:], in_=wo_d[:].rearrange("c p m -> p c m"))
            for m in range(TB):
                cts = ct_pool.tile([128, HL, 128], BF16)
                nc.sync.dma_start(
                    out=cts[:],
                    in_=ct_d[:, :, m * 128 : (m + 1) * 128].rearrange(
                        "c p m2 -> p c m2"
                    ),
                )
                osb = out_pool.tile([128, D], F32)
                for n in range(4):
                    ops = psum_p3.tile([128, 512], F32)
                    for c in range(HL):
                        nc.tensor.matmul(
                            ops[:],
                            cts[:, c, :],
                            wo_sb[:, c, n * 512 : (n + 1) * 512],
                            start=(c == 0),
                            stop=(c == HL - 1),
                        )
                    nc.vector.tensor_copy(out=osb[:, n * 512 : (n + 1) * 512], in_=ops[:])
                nc.sync.dma_start(out=out_d[m * 128 : (m + 1) * 128, :], in_=osb[:])

    nc.finalize()
    return nc


def _get_nc():
    if "nc" not in _NC_CACHE:
        _NC_CACHE["nc"] = build_nc()
    return _NC_CACHE["nc"]


def _rope_tables():
    inv_freq = 1.0 / (ROPE_THETA ** (np.arange(0, DH, 2, dtype=np.float32) / DH))
    freqs = np.arange(S, dtype=np.float32)[:, None] * inv_freq[None, :]
    emb = np.concatenate([freqs, freqs], axis=-1)  # [S, 128]
    cosT = np.ascontiguousarray(np.cos(emb).T.astype(np.float32))  # [128, S]
    sinS = np.sin(emb).T.astype(np.float32).copy()
    sinS[0:64, :] *= -1.0  # sign-folded rotate_half
    return cosT, np.ascontiguousarray(sinS)


def kernel(x, wq, bq, wk, bk, wv, bv, wo, bo, _trace=False, _tmpdir=None):
    x = np.asarray(x, dtype=np.float32)
    wq = np.asarray(wq, dtype=np.float32)
    wk = np.asarray(wk, dtype=np.float32)
    wv = np.asarray(wv, dtype=np.float32)
    wo = np.asarray(wo, dtype=np.float32)
    bq = np.asarray(bq, dtype=np.float32)
    bk = np.asarray(bk, dtype=np.float32)
    bv = np.asarray(bv, dtype=np.float32)
    bo = np.asarray(bo, dtype=np.float32)

    nc = _get_nc()
    cosT, sinS = _rope_tables()

    def qk_pack(w, g):
        ws = w[g * 1024 : (g + 1) * 1024, :]
        return np.ascontiguousarray(
            ws.reshape(HL, 128, KO, 128).transpose(0, 2, 3, 1).astype(NPBF)
        )

    packs = []
    for g in range(2):
        wv_s = wv[g * 1024 : (g + 1) * 1024, :]
        wv_p = np.ascontiguousarray(
            wv_s.reshape(HL * DH, KO, 128).transpose(1, 2, 0).astype(NPBF)
        )
        wo_s = wo[:, g * 1024 : (g + 1) * 1024]
        wo_p = np.ascontiguousarray(
            wo_s.reshape(D, HL, 128).transpose(1, 2, 0).astype(NPBF)
        )
        bq_p = np.ascontiguousarray(
            (bq[g * 1024 : (g + 1) * 1024] * QSCALE).reshape(HL, 128).T
        )
        bk_p = np.ascontiguousarray(bk[g * 1024 : (g + 1) * 1024].reshape(HL, 128).T)
        packs.append(
            dict(
                wq=qk_pack(wq, g),
                wk=qk_pack(wk, g),
                wv=wv_p,
                wo=wo_p,
                bq=bq_p,
                bk=bk_p,
            )
        )

    in_maps = []
    xts = [
        np.ascontiguousarray(x[b].T.astype(NPBF)).reshape(KO, 128, S)
        for b in range(B)
    ]
    for c in range(8):
        b, g = c // 2, c % 2
        m = dict(packs[g])
        m["xt"] = xts[b]
        m["cosT"] = cosT
        m["sinS"] = sinS
        in_maps.append(m)

    res = run_bass_kernel_spmd(
        nc,
        in_maps,
        core_ids=list(range(8)),
        trace=_trace,
        tmpdir=_tmpdir,
    )

    bo_eff = bo + wo @ bv
    out = np.empty((B, S, D), dtype=np.float32)
    for b in range(B):
        out[b] = res.results[2 * b]["out"] + res.results[2 * b + 1]["out"]
        out[b] += bo_eff[None, :]
    if _trace:
        kernel.last_result = res
    return out

